# revision 11
# baseline (speedup 1.0000x reference)
"""Trainium2 Bass kernel for MixedIntQuantizedLinear.

Computation (see reference):
  W_dq[o,i] = W_int[o,i] * (scale_i32[o, i//64] / 2^24)
  per-token: amax_t = clip(max|x_t|, 1e-8); s_t = amax_t/127
             q_t = round(x_t / s_t)  (|q| <= 127, round-to-nearest-even)
  y[t,o] = s_t * sum_i q_t[i] * W_dq[o,i] + bias[o]

Sharding over 8 NeuronCores: 2 token-groups (batch halves) x 4
out-feature groups of 1024.  Each core computes y_core [4096, 1024].

v10 strategy:
  - W is dequantized on host (W_int * block_scale -> bf16) and shipped
    in per-partition k-major layout [128, KT*O_CORE], so the full
    8.4MB W loads with just EIGHT 1MB DMAs straight into the resident
    wt tensors (the Tile scheduler has only 8 DMA-completion semaphore
    lanes shared by ALL queues; many small DMAs serialize across
    queues).  Zero device-side dequant.
  - One matmul per k-tile with the full 1024-wide moving operand
    (bf16 moving max is 128x1024): 32 matmuls/token-tile into a
    2-bank PSUM tile, halving instruction count and LDWEIGHTS.
  - Engine split: x loads + qT transposes on sync queue, y stores +
    activations on scalar, W loads on gpsimd, quant math on DVE.
  - x tiles [128 tok, 4096]: absmax-reduce; quantize via the fp32
    magic-number trick (x*inv + 1.5*2^23 rounds to int with fp32 RNE);
    ScalarE subtracts the magic and emits exact-integer bf16; one XBAR
    DMA-transpose produces qT [128, 32, 128].
  - Epilogue: one fused DVE scalar_tensor_tensor (psum*s_t + bias).
  - Phase 1: first EARLY=3 tiles' matmuls interleave with the 8 W
    group DMAs (4 k-stripes per group) so the PE consumes W as it
    lands; phase-1 tiles use 2-chunk loads/quant/transposes to cut
    the x->first-matmul latency.
"""

import os
import sys

sys.path.insert(0, "/opt/trn_rl_repo")

import numpy as np

import concourse.bass as bass
import concourse.tile as tile
from concourse import bacc, mybir
from concourse.bass_utils import run_bass_kernel_spmd

P = 128
IN_F = 4096
OUT_F = 4096
TOKENS = 8192          # 4 * 2048
N_CORES = 8
TG = 2                 # token groups
OG = 4                 # out-feature groups
T_CORE = TOKENS // TG  # 4096 tokens per core
O_CORE = OUT_F // OG   # 1024 out features per core
KT = IN_F // P         # 32 contraction tiles (= W k-stripes)
TT = T_CORE // P       # 32 token tiles
MAGIC = 12582912.0     # 1.5 * 2^23: fp32 round-to-int magic constant
SCALE_SHIFT = 24
QK = 8                 # k-tiles per wt quarter tensor
WG = 8                 # k-stripes per W group DMA (one quarter)

F32 = mybir.dt.float32
BF16 = mybir.dt.bfloat16
ACT_COPY = mybir.ActivationFunctionType.Copy

EARLY = 3  # token tiles whose matmuls interleave with W-group arrival


def build_kernel():
    nc = bacc.Bacc(None, target_bir_lowering=False, debug=False)

    x_d = nc.dram_tensor("x", [T_CORE, IN_F], F32, kind="ExternalInput")
    # per-partition k-major W: w_d[p, k*O_CORE + o] = W_dq[o, k*128+p]
    w_d = nc.dram_tensor("w", [P, KT * O_CORE], BF16, kind="ExternalInput")
    b_d = nc.dram_tensor("b", [1, O_CORE], F32, kind="ExternalInput")
    y_d = nc.dram_tensor("y", [T_CORE, O_CORE], F32, kind="ExternalOutput")

    XIN_BUFS = int(os.environ.get("KERNEL_XIN", "3"))
    QB_BUFS = int(os.environ.get("KERNEL_QB", "2"))
    QT_BUFS = int(os.environ.get("KERNEL_QT", "3"))
    OROW_BUFS = int(os.environ.get("KERNEL_OROW", "2"))
    PSUM_BUFS = int(os.environ.get("KERNEL_PSUM", "6"))

    with tile.TileContext(nc) as tc:
        with (
            tc.tile_pool(name="const", bufs=1) as const_pool,
            tc.tile_pool(name="wt", bufs=1) as wt_pool,
            tc.tile_pool(name="psum_y", bufs=PSUM_BUFS, space="PSUM") as psum_y,
            tc.tile_pool(name="psum_misc", bufs=2, space="PSUM") as psum_misc,
        ):
            # W quarter tensors [128, QK, O_CORE]; group g (4 stripes, 1MB)
            # lands in wtq[g//2][:, (g%2)*WG:(g%2+1)*WG, :] via one DMA from
            # the contiguous dram slice.
            wtq = [
                wt_pool.tile([P, QK, O_CORE], BF16, name=f"wt{q}", tag=f"wt{q}")
                for q in range(KT // QK)
            ]

            def emit_w_group(g, eng):
                # one full quarter per DMA; quarters split between the ACT
                # HWDGE ring and the SWDGE queue so W streams on two queues
                # concurrently with x on the SP ring (each queue only
                # sustains ~120-180GB/s; three streams saturate HBM)
                eng.dma_start(
                    wtq[g][:],
                    w_d[:, g * WG * O_CORE:(g + 1) * WG * O_CORE])

            with (
                tc.tile_pool(name="xin", bufs=XIN_BUFS) as xin_pool,
                tc.tile_pool(name="small", bufs=6) as small_pool,
                tc.tile_pool(name="qb", bufs=QB_BUFS) as qb_pool,
                tc.tile_pool(name="qt", bufs=QT_BUFS) as qt_pool,
                tc.tile_pool(name="orow", bufs=OROW_BUFS) as orow_pool,
            ):
                # phase-1 tiles load in 2 chunks to cut first-quant latency
                NCH = 2
                CW = IN_F // NCH
                early_x = {}

                def emit_x_chunks(tt):
                    xt = xin_pool.tile([P, IN_F], F32, tag="xt")
                    for c in range(NCH):
                        nc.sync.dma_start(
                            xt[:, c * CW:(c + 1) * CW],
                            x_d[tt * P:(tt + 1) * P, c * CW:(c + 1) * CW])
                    early_x[tt] = xt

                # Phase-1 streaming plan: SP ring carries x (and later the
                # transposes, which cost no HBM -- XBAR is SBUF fabric); the
                # ACT ring carries bias + W0-W2; W3 (needed last) trickles on
                # the slow SWDGE queue as a third HBM stream.
                ones_k1 = const_pool.tile([1, P], F32)
                nc.vector.memset(ones_k1[:], 1.0)
                bias_sb = const_pool.tile([1, O_CORE], F32)
                nc.scalar.dma_start(bias_sb[:], b_d[:])

                xt0 = xin_pool.tile([P, IN_F], F32, tag="xt")
                nc.sync.dma_start(xt0[:, :CW], x_d[0:P, :CW])
                nc.sync.dma_start(xt0[:, CW:], x_d[0:P, CW:])
                early_x[0] = xt0
                emit_w_group(0, nc.scalar)
                emit_w_group(3, nc.gpsimd)
                xt1 = xin_pool.tile([P, IN_F], F32, tag="xt")
                nc.sync.dma_start(xt1[:], x_d[P:2 * P, :])
                early_x[1] = xt1
                emit_w_group(1, nc.scalar)
                emit_w_group(2, nc.scalar)

                # ---- bias broadcast row -> [128, O_CORE] via K=1 matmul
                bias_bcast = const_pool.tile([P, O_CORE], F32)
                for oc in range(2):
                    pb = psum_misc.tile([P, 512], F32, tag="pb")
                    nc.tensor.matmul(
                        pb[:], ones_k1[:], bias_sb[:, oc * 512:(oc + 1) * 512],
                        start=True, stop=True,
                    )
                    nc.scalar.copy(bias_bcast[:, oc * 512:(oc + 1) * 512], pb[:])

                xts = {}

                def emit_x_load(tt):
                    if tt >= TT or tt in xts:
                        return
                    xt = xin_pool.tile([P, IN_F], F32, tag="xt")
                    nc.sync.dma_start(xt[:], x_d[tt * P:(tt + 1) * P, :])
                    xts[tt] = xt

                quant = {}   # tt -> (qt, s_t)

                def emit_quant_chunked(tt, nch):
                    # chunk-pipelined quant for phase-1 tiles
                    xt = early_x.pop(tt)
                    cw = IN_F // nch
                    if nch > 1:
                        am = small_pool.tile([P, nch], F32, tag="am4")
                        for c in range(nch):
                            nc.vector.tensor_reduce(
                                am[:, c:c + 1], xt[:, c * cw:(c + 1) * cw],
                                axis=mybir.AxisListType.X,
                                op=mybir.AluOpType.max,
                                apply_absolute_value=True,
                            )
                        amax = small_pool.tile([P, 1], F32, tag="amax")
                        nc.vector.tensor_reduce(
                            amax[:], am[:], axis=mybir.AxisListType.X,
                            op=mybir.AluOpType.max,
                        )
                    else:
                        amax = small_pool.tile([P, 1], F32, tag="amax")
                        nc.vector.tensor_reduce(
                            amax[:], xt[:], axis=mybir.AxisListType.X,
                            op=mybir.AluOpType.max,
                            apply_absolute_value=True,
                        )
                    nc.vector.tensor_scalar_max(amax[:], amax[:], 1e-8)
                    s_t = small_pool.tile([P, 1], F32, tag="s_t")
                    nc.vector.tensor_scalar_mul(s_t[:], amax[:], 1.0 / 127.0)
                    inv = small_pool.tile([P, 1], F32, tag="inv")
                    nc.vector.reciprocal(inv[:], s_t[:])
                    qb = qb_pool.tile([P, IN_F], BF16, tag="qb")
                    qt = qt_pool.tile([P, KT, P], BF16, tag="qt")
                    for c in range(nch):
                        sl = slice(c * cw, (c + 1) * cw)
                        nc.vector.tensor_scalar(
                            xt[:, sl], xt[:, sl], inv[:], MAGIC,
                            op0=mybir.AluOpType.mult,
                            op1=mybir.AluOpType.add,
                        )
                        nc.scalar.activation(qb[:, sl], xt[:, sl], ACT_COPY,
                                             bias=-MAGIC)
                        nc.sync.dma_start_transpose(
                            qt[:, c * (KT // nch):(c + 1) * (KT // nch), :],
                            qb[:, sl])
                    quant[tt] = (qt, s_t)

                def emit_quant(tt):
                    xt = xts.pop(tt)
                    amax = small_pool.tile([P, 1], F32, tag="amax")
                    nc.vector.tensor_reduce(
                        amax[:], xt[:], axis=mybir.AxisListType.X,
                        op=mybir.AluOpType.max, apply_absolute_value=True,
                    )
                    nc.vector.tensor_scalar_max(amax[:], amax[:], 1e-8)
                    s_t = small_pool.tile([P, 1], F32, tag="s_t")
                    nc.vector.tensor_scalar_mul(s_t[:], amax[:], 1.0 / 127.0)
                    inv = small_pool.tile([P, 1], F32, tag="inv")
                    nc.vector.reciprocal(inv[:], s_t[:])

                    # x <- x * inv + MAGIC  (fp32; integer part = q + MAGIC)
                    nc.vector.tensor_scalar(
                        xt[:], xt[:], inv[:], MAGIC,
                        op0=mybir.AluOpType.mult, op1=mybir.AluOpType.add,
                    )
                    # q (exact small ints) in bf16
                    qb = qb_pool.tile([P, IN_F], BF16, tag="qb")
                    nc.scalar.activation(qb[:], xt[:], ACT_COPY, bias=-MAGIC)

                    # XBAR transpose -> qT [128(i), KT, 128(t)]
                    qt = qt_pool.tile([P, KT, P], BF16, tag="qt")
                    nc.sync.dma_start_transpose(qt[:], qb[:])
                    quant[tt] = (qt, s_t)

                def emit_mm_k(tt, k, pys):
                    qt, _ = quant[tt]
                    for oc in range(2):
                        nc.tensor.matmul(
                            pys[(tt, oc)][:], qt[:, k, :],
                            wtq[k // QK][:, k % QK,
                                         oc * 512:(oc + 1) * 512],
                            start=(k == 0), stop=(k == KT - 1),
                        )

                def emit_epilogue(tt, pys):
                    _, s_t = quant[tt]
                    orow = orow_pool.tile([P, O_CORE], F32, tag="orow")
                    for oc in range(2):
                        py = pys.pop((tt, oc))
                        nc.vector.scalar_tensor_tensor(
                            orow[:, oc * 512:(oc + 1) * 512], py[:], s_t[:],
                            bias_bcast[:, oc * 512:(oc + 1) * 512],
                            op0=mybir.AluOpType.mult,
                            op1=mybir.AluOpType.add,
                        )
                    del quant[tt]
                    nc.scalar.dma_start(y_d[tt * P:(tt + 1) * P, :], orow[:])

                # ---- phase 1: W groups with first EARLY tiles' matmuls
                # interleaved per group
                emit_quant_chunked(0, NCH)
                emit_quant_chunked(1, 1)
                xt2 = xin_pool.tile([P, IN_F], F32, tag="xt")
                nc.sync.dma_start(xt2[:], x_d[2 * P:3 * P, :])
                early_x[2] = xt2
                emit_quant_chunked(2, 1)

                pys = {}
                for tt in range(EARLY):
                    for oc in range(2):
                        pys[(tt, oc)] = psum_y.tile(
                            [P, 512], F32, tag="py", name=f"py_{tt}_{oc}")
                for g in range(KT // WG):
                    for tt in range(EARLY):
                        for k in range(g * WG, (g + 1) * WG):
                            emit_mm_k(tt, k, pys)
                for tt in range(EARLY):
                    emit_epilogue(tt, pys)

                # ---- steady state ----
                emit_x_load(EARLY)
                emit_x_load(EARLY + 1)
                for tt in range(EARLY, TT):
                    emit_x_load(tt + 2)
                    emit_quant(tt)
                    tpys = {}
                    for oc in range(2):
                        tpys[(tt, oc)] = psum_y.tile(
                            [P, 512], F32, tag="py", name=f"py_{tt}_{oc}")
                    for k in range(KT):
                        emit_mm_k(tt, k, tpys)
                    emit_epilogue(tt, tpys)

    nc.compile()
    return nc


_NC_CACHE = None


def _get_nc():
    global _NC_CACHE
    if _NC_CACHE is None:
        _NC_CACHE = build_kernel()
    return _NC_CACHE


def kernel(x, W_int, scale_i32, bias, _trace=False, _tmpdir=None):
    import ml_dtypes

    nc = _get_nc()
    x2 = np.asarray(x, dtype=np.float32).reshape(TOKENS, IN_F)
    # host-side dequant: W_dq = W_int * (scale/2^24), bf16
    sc = np.asarray(scale_i32, dtype=np.int32).astype(np.float32) * (
        1.0 / (1 << SCALE_SHIFT))
    W_dq = np.asarray(W_int, dtype=np.int32).astype(np.float32) * np.repeat(
        sc, 64, axis=1)
    # per-partition k-major device layout: W_dev[og][p, k*O_CORE+o]
    #   = W_dq[og*O_CORE+o, k*128+p]
    W_bf = W_dq.astype(ml_dtypes.bfloat16)  # [OUT_F, IN_F]
    bias2 = np.asarray(bias, dtype=np.float32).reshape(1, OUT_F)

    in_maps = []
    for c in range(N_CORES):
        tg, og = c // OG, c % OG
        wo = W_bf[og * O_CORE:(og + 1) * O_CORE, :]       # [O_CORE, IN_F]
        wdev = np.ascontiguousarray(
            wo.reshape(O_CORE, KT, P).transpose(2, 1, 0).reshape(
                P, KT * O_CORE))
        in_maps.append({
            "x": np.ascontiguousarray(x2[tg * T_CORE:(tg + 1) * T_CORE]),
            "w": wdev,
            "b": np.ascontiguousarray(bias2[:, og * O_CORE:(og + 1) * O_CORE]),
        })

    res = run_bass_kernel_spmd(
        nc, in_maps, core_ids=list(range(N_CORES)),
        trace=_trace, tmpdir=_tmpdir,
    )
    y = np.empty((TOKENS, OUT_F), dtype=np.float32)
    for c in range(N_CORES):
        tg, og = c // OG, c % OG
        y[tg * T_CORE:(tg + 1) * T_CORE, og * O_CORE:(og + 1) * O_CORE] = \
            res.results[c]["y"]
    out = y.reshape(4, 2048, OUT_F)
    if _trace:
        return out, res
    return out
